# revision 27
# baseline (speedup 1.0000x reference)
"""Trainium2 Bass kernel for batched multi-head self-attention.

Problem: x[8,1024,768], w_qkv[768,2304], b_qkv[2304] ->
         out[8,1024,768]  (12 heads, head_dim 64, scale 768**-0.5)

Sharding: data-parallel over batch; each of the 8 NeuronCores processes one
batch element end-to-end (no collectives).

v2 design (all fp16 inputs to the PE, f32 PSUM accumulation):
  - Everything streams in [feature, token] / [k-token, q-token] orientation;
    the output is produced transposed (oT[d, n]) and the host transposes it
    back, eliminating all on-device PE transposes of the baseline.
  - QK projection -> qkt e-tiles [128 feat, 1024 tok] fp16 (pair-grouped
    feature permutation on the host); V projection -> vp[kt] [128 tok, 780]
    fp16 laid out per head as [V_h | 1] (ones column gives softmax
    denominators for free in the PV matmul).
  - Attention per head pair: energy^T[k,q] (K=64 contraction, two heads in
    row halves), ScalarE exp (fused *scale) PSUM->SBUF fp16, PV matmul
    with stationary [V|1] accumulating over the 8 k-chunks -> [65, 512]
    PSUM (row 64 = denominator).
  - Normalize without transposing: DVE reciprocal of the denominator row,
    PE ones-matmul broadcasts it across 64 partitions, DVE multiply ->
    oT tile, DMA out.
  - Software pipelining: the ScalarE exp stream (~100us/core) is the
    near-critical path; energies of pair p, PV of pair p-1, and the QK
    projection of pair p+1 are interleaved in the PE queue so the PE never
    waits on ScalarE.
"""

import numpy as np

import concourse.mybir as mybir
import concourse.tile as tile
from concourse import bacc
from concourse.bass_utils import run_bass_kernel_spmd

B, NT, D, H, HD = 8, 1024, 768, 12, 64
KC = D // 128           # 6 contraction chunks
NPAIR = H // 2          # 6 head pairs
SCALE = float(D) ** -0.5
F32 = mybir.dt.float32
F32R = mybir.dt.float32r
FP16 = mybir.dt.float16
HW65 = HD + 1           # 65: per-head [V | 1]
V_W = H * HW65          # 780
HW390 = 6 * HW65        # 390: six heads of [V | 1]
VSTRIDE = 128           # per-head stride in vp (padded for FWL)


def _build():
    nc = bacc.Bacc("TRN2", target_bir_lowering=False, debug=False, num_devices=B)

    xT16 = nc.dram_tensor("xT16", [D, NT], FP16, kind="ExternalInput")
    wqk = nc.dram_tensor("wqk", [D, 2 * D], FP16, kind="ExternalInput")
    wv = nc.dram_tensor("wv", [D, V_W], FP16, kind="ExternalInput")
    bqk = nc.dram_tensor("bqk", [128, H], F32, kind="ExternalInput")
    bv = nc.dram_tensor("bv", [128, V_W], FP16, kind="ExternalInput")
    # unnormalized out^T with per-head denominator rows (row h*65+64);
    # the host divides + transposes
    oT = nc.dram_tensor("oT", [H * HW65, NT], FP16, kind="ExternalOutput")

    with tile.TileContext(nc) as tc:
        with (
            tc.tile_pool(name="res", bufs=1) as res,          # persistent tensors
            tc.tile_pool(name="wstream", bufs=2) as wstream,  # streamed QK weights
            tc.tile_pool(name="expp", bufs=26) as expp,       # exp tiles
            tc.tile_pool(name="osb", bufs=4) as osb,          # output staging
            tc.tile_pool(name="eps", bufs=2, space="PSUM") as epsp,    # 2x2 banks
            tc.tile_pool(name="pvs", bufs=2, space="PSUM") as pvs,     # 2x1 banks
            tc.tile_pool(name="proj", bufs=2, space="PSUM") as projp,  # 2x1 banks
        ):
            # ---- persistent SBUF tensors ----
            xt16a = res.tile([128, KC, NT], FP16, tag="xt16", name="xt16")
            xt16 = [xt16a[:, k, :] for k in range(KC)]
            qkt = [res.tile([128, NT], FP16, tag=f"qkt{e}", name=f"qkt{e}") for e in range(H)]
            vp = [res.tile([128, V_W], FP16, tag=f"vp{t}", name=f"vp{t}") for t in range(8)]
            wv_a = res.tile([128, KC, V_W], FP16, tag="wv", name="wv")
            wv_sb = [wv_a[:, k, :] for k in range(KC)]
            bqk_sb = res.tile([128, H], F32, tag="bqk")
            bv_sb = res.tile([128, V_W], FP16, tag="bv")

            def dma_wqk(p):
                ta = wstream.tile([128, KC, 256], FP16, tag="wqk", name=f"wqk_{p}")
                nc.sync.dma_start(
                    ta[:],
                    wqk[:, p * 256:(p + 1) * 256]
                    .rearrange("(k r) f -> r k f", k=KC))
                return [ta[:, k, :] for k in range(KC)]

            # priority DMAs, dependency-ordered halves: the first QK chain
            # (i=0, tcn=0) starts once the first two transfers land
            wqk0 = wstream.tile([128, KC, 256], FP16, tag="wqk", name="wqk_0")
            wqk_t = [wqk0[:, k, :] for k in range(KC)]
            for ih in range(2):
                nc.sync.dma_start(
                    wqk0[:, :, ih * 128:(ih + 1) * 128],
                    wqk[:, ih * 128:ih * 128 + 128]
                    .rearrange("(k r) f -> r k f", k=KC))
                nc.sync.dma_start(
                    xt16a[:, :, ih * 512:(ih + 1) * 512],
                    xT16[:, ih * 512:(ih + 1) * 512]
                    .rearrange("(k r) t -> r k t", k=KC))
            nc.sync.dma_start(bqk_sb[:], bqk[:, :])
            nc.sync.dma_start(bv_sb[:], bv[:, :])
            nc.sync.dma_start(
                wv_a[:], wv[:, :].rearrange("(k r) f -> r k f", k=KC))

            # PE warmup while DMAs stream: ramps the HAM clock gate to 2.4GHz
            warm = res.tile([128, 512], FP16, tag="warm", name="warm")
            nc.vector.memset(warm[:], 0.0)
            for wi in range(24):
                wps = projp.tile([128, 512], F32, tag="proj", name="warmps")
                nc.tensor.matmul(wps[:], warm[:, 0:128], warm[:],
                                 start=True, stop=True)

            def qk_chain(p, wt, i, tcn):
                """One QK projection chain: e-tile half (i) x token half (tcn)."""
                et = 2 * p + i
                ps = projp.tile([128, 512], F32, tag="proj", name="psqk")
                for k in range(KC):
                    nc.tensor.matmul(
                        ps[:],
                        wt[k][:, i * 128:(i + 1) * 128],
                        xt16[k][:, tcn * 512:(tcn + 1) * 512],
                        start=(k == 0), stop=(k == KC - 1))
                nc.vector.tensor_scalar_add(
                    qkt[et][:, tcn * 512:(tcn + 1) * 512],
                    ps[:], bqk_sb[:, et:et + 1])

            def vproj_unit(t, n):
                """V projection for token chunk t, head group n (6 heads)."""
                ps = projp.tile([128, 512], F32, tag="proj", name="psv")
                for k in range(KC):
                    nc.tensor.matmul(ps[:, 0:HW390],
                                     xt16[k][:, t * 128:(t + 1) * 128],
                                     wv_sb[k][:, n * HW390:(n + 1) * HW390],
                                     start=(k == 0), stop=(k == KC - 1))
                nc.vector.tensor_add(
                    vp[t][:, n * HW390:(n + 1) * HW390],
                    ps[:, 0:HW390],
                    bv_sb[:, n * HW390:(n + 1) * HW390])

            def energy_exp(p, qc, kt):
                """Energy^T for both heads of pair p, k-chunk kt, q-half qc,
                then exp -> fp16 SBUF tile [128, head-A 512 | head-B 512]."""
                eps = epsp.tile([128, 1024], F32, tag="eps", name="eps")
                for i in range(2):
                    rows = slice(i * HD, (i + 1) * HD)
                    nc.tensor.matmul(
                        eps[:, i * 512:(i + 1) * 512],
                        qkt[2 * p + 1][rows, kt * 128:(kt + 1) * 128],
                        qkt[2 * p][rows, qc * 512:(qc + 1) * 512],
                        start=True, stop=True)
                ex = expp.tile([128, 1024], FP16, tag="exp", name="ex")
                nc.scalar.activation(ex[:], eps[:],
                                     mybir.ActivationFunctionType.Exp,
                                     bias=0.0, scale=SCALE)
                return ex

            def pv_mm(pvp, h, ex, i, kt):
                nc.tensor.matmul(
                    pvp[:],
                    vp[kt][:, h * HW65:(h + 1) * HW65],
                    ex[:, i * 512:(i + 1) * 512],
                    start=(kt == 0), stop=(kt == 7))

            ot_store = {}

            def normalize_out(pvp, h, qc):
                """pvp [65, 512]: rows 0-63 unnormalized out^T, row 64 denom.
                Stage to SBUF fp16; one DMA per head; the host divides."""
                if qc == 0:
                    ot_store[h] = osb.tile([HW65, NT], FP16, tag="ot",
                                           name=f"ot{h % 4}")
                ot = ot_store[h]
                with nc.allow_low_precision(reason="host renormalizes in f32"):
                    nc.vector.tensor_copy(ot[:, qc * 512:(qc + 1) * 512], pvp[:])
                if qc == 1:
                    nc.scalar.dma_start(
                        oT[h * HW65:(h + 1) * HW65, :], ot[:])

            # ---------------- pair 0: projections + energies ----------------
            for i in range(2):
                for tcn in range(2):
                    qk_chain(0, wqk_t, i, tcn)
            wqk_next = dma_wqk(1)

            ex_store = {}  # (p, qc, kt) -> ex tile (only cur/prev pair live)
            vunits = [(t, n) for n in range(2) for t in range(8)]
            vi = 0
            ci0 = 0
            for qc in range(2):
                for kt in range(8):
                    ex_store[(0, qc, kt)] = energy_exp(0, qc, kt)
                    # interleave one V-projection unit per energy slot, plus
                    # pair-1 QK projection every 4th slot
                    if vi < 16:
                        vproj_unit(*vunits[vi]); vi += 1
                    if kt % 4 == 3 and ci0 < 4:
                        qk_chain(1, wqk_next, ci0 // 2, ci0 % 2); ci0 += 1
            while vi < 16:
                vproj_unit(*vunits[vi]); vi += 1
            while ci0 < 4:
                qk_chain(1, wqk_next, ci0 // 2, ci0 % 2); ci0 += 1
            wqk_next = dma_wqk(2)

            # ---------------- steady state: pairs 1..5 ----------------
            for p in range(1, NPAIR):
                chains = [(i, tcn) for i in range(2) for tcn in range(2)]
                ci = 0 if p < NPAIR - 1 else 4  # no pair p+1 to project at p=5
                for qc in range(2):
                    # PV accumulators of pair p-1, this q-half
                    pva = pvs.tile([HW65, 512], F32, tag="pv", name="pva")
                    pvb = pvs.tile([HW65, 512], F32, tag="pv", name="pvb")
                    for kt in range(8):
                        exm = ex_store[(p - 1, qc, kt)]
                        pv_mm(pva, 2 * (p - 1), exm, 0, kt)
                        pv_mm(pvb, 2 * (p - 1) + 1, exm, 1, kt)
                        ex_store[(p, qc, kt)] = energy_exp(p, qc, kt)
                        if kt % 2 == 1 and ci < 4:  # one QK chain per 2 slots
                            qk_chain(p + 1, wqk_next, *chains[ci]); ci += 1
                    while ci < 4:
                        qk_chain(p + 1, wqk_next, *chains[ci]); ci += 1
                    normalize_out(pva, 2 * (p - 1), qc)
                    normalize_out(pvb, 2 * (p - 1) + 1, qc)
                    for kt in range(8):
                        del ex_store[(p - 1, qc, kt)]
                if p + 2 < NPAIR:
                    wqk_next = dma_wqk(p + 2)

            # ---------------- tail: PV + output for pair 5 ----------------
            for qc in range(2):
                pva = pvs.tile([HW65, 512], F32, tag="pv", name="pva")
                pvb = pvs.tile([HW65, 512], F32, tag="pv", name="pvb")
                for kt in range(8):
                    exm = ex_store[(NPAIR - 1, qc, kt)]
                    pv_mm(pva, 2 * (NPAIR - 1), exm, 0, kt)
                    pv_mm(pvb, 2 * (NPAIR - 1) + 1, exm, 1, kt)
                normalize_out(pva, 2 * (NPAIR - 1), qc)
                normalize_out(pvb, 2 * (NPAIR - 1) + 1, qc)

    nc.compile()
    return nc


_NC_CACHE = None


def _get_nc():
    global _NC_CACHE
    if _NC_CACHE is None:
        _NC_CACHE = _build()
    return _NC_CACHE


def _perm_indices():
    d3 = np.arange(HD) * 3
    qk_cols = []
    for p in range(NPAIR):
        for s in (0, 1):  # Q tile then K tile
            for h in (2 * p, 2 * p + 1):
                qk_cols.append(h * (HD * 3) + d3 + s)
    v_cols = [h * (HD * 3) + d3 + 2 for h in range(H)]
    return np.concatenate(qk_cols), np.concatenate(v_cols)


def make_in_maps(x, w_qkv, b_qkv):
    qk_idx, v_idx = _perm_indices()
    w = np.asarray(w_qkv, dtype=np.float32)
    b = np.asarray(b_qkv, dtype=np.float32)
    wqk = np.ascontiguousarray(w[:, qk_idx], dtype=np.float16)
    # [D, 780]: per head [V_h (64 cols) | zero col]; matching bias gets 1.0 in
    # the zero col so vp = x@wv + bv carries softmax-denominator ones
    wv = np.zeros((D, V_W), dtype=np.float16)
    bv1 = np.zeros(V_W, dtype=np.float16)
    wv_perm = w[:, v_idx]
    bv_perm = b[v_idx]
    for h in range(H):
        wv[:, h * HW65:h * HW65 + HD] = wv_perm[:, h * HD:(h + 1) * HD]
        bv1[h * HW65:h * HW65 + HD] = bv_perm[h * HD:(h + 1) * HD]
        bv1[h * HW65 + HD] = 1.0
    bqk = np.ascontiguousarray(b[qk_idx].reshape(H, 128).T)
    bv = np.ascontiguousarray(np.broadcast_to(bv1, (128, V_W)))
    return [
        {
            "xT16": np.ascontiguousarray(np.asarray(x[bi], dtype=np.float16).T),
            "wqk": wqk, "wv": wv, "bqk": bqk, "bv": bv,
        }
        for bi in range(B)
    ]


def kernel(x, w_qkv, b_qkv):
    nc = _get_nc()
    in_maps = make_in_maps(x, w_qkv, b_qkv)
    res = run_bass_kernel_spmd(nc, in_maps, core_ids=list(range(B)))
    return np.stack([_finish(res.results[b]) for b in range(B)])


def _finish(r):
    """Divide the unnormalized out^T by the softmax denominators, transpose."""
    oT = np.asarray(r["oT"], dtype=np.float32).reshape(H, HW65, NT)
    return np.ascontiguousarray(
        (oT[:, :HD, :] / oT[:, HD:HW65, :]).reshape(D, NT).T)
